# revision 4
# baseline (speedup 1.0000x reference)
"""Trainium2 Bass kernel for nn_CraftaxAgent (encoder MLP + done-masked LSTM + heads).

Data-parallel over batch B=256 across 8 NeuronCores (32 batch rows per core).
The time scan (T=128) runs locally per core, pipelined behind the encoder.

Layout strategy: the encoder runs "feature-major" (transposed) so the
contraction dimension is always on PE partitions -- the host hands each core x
pre-transposed as [OBS_padded, T*B_local] and no on-device transpose of the big
x tensor is ever needed.  The LSTM gates are computed batch-major
[B=32 partitions, 4H=128 free] so gate slices are free-dim columns at a common
base partition (a DVE requirement), the done-mask applies as a per-partition
tensor_scalar, and the gate bias rides in the matmul via a ones-row augmented
h3.  One DVE 32x32 stream-transpose per step returns the state to [H, B] for
the recurrent matmul; a second one accumulates hseq feature-major for the
output head.  Gates are reordered (i, f, o, g) on the host so one sigmoid
covers columns 0:96 and one tanh covers 96:128.

Self-contained: hardcodes all shapes; no file reads.
"""

import functools

import numpy as np

import concourse.bass as bass
import concourse.bacc as bacc
import concourse.tile as tile
from concourse import mybir
from concourse.bass_utils import run_bass_kernel_spmd

# Problem shape (hardcoded per spec)
OBS = 8268
A = 43
T = 128
B = 256
H = 32
G = 4 * H          # 128 gate columns
AO = A + 1         # 44 output cols (logits + value)

NCORES = 8
BL = B // NCORES   # 32 batch rows per core
N = T * BL         # 4096 rows per core
KT = 65            # K tiles of 128 over OBS (padded)
OBSP = KT * 128    # 8320
NCH = 8            # column chunks per core
CW = N // NCH      # 512 cols per chunk
TPC = T // NCH     # 16 time steps per chunk

f32 = mybir.dt.float32

# Layer-1 matmul dtype for x / W1: "f32" (exact) or "f16" (half DMA + 4x PE rate)
DT_X = "f16"
_DTX_MY = {"f32": f32, "f16": mybir.dt.float16}
_DTX_NP = {"f32": np.float32, "f16": np.float16}


def _build(dt_x: str, reps: int = 1):
    """Build + compile the SPMD program (identical on all cores)."""
    dtx = _DTX_MY[dt_x]
    nc = bacc.Bacc("TRN2", target_bir_lowering=False, debug=False)

    xt = nc.dram_tensor("xt", [OBSP, N], dtx, kind="ExternalInput").ap()
    maskc = nc.dram_tensor("maskc", [BL, T], f32, kind="ExternalInput").ap()
    h0b = nc.dram_tensor("h0b", [BL, H], f32, kind="ExternalInput").ap()
    c0b = nc.dram_tensor("c0b", [BL, H], f32, kind="ExternalInput").ap()
    w1 = nc.dram_tensor("w1", [128, KT * 64], dtx, kind="ExternalInput").ap()
    b1 = nc.dram_tensor("b1", [64, 1], f32, kind="ExternalInput").ap()
    w2t = nc.dram_tensor("w2t", [64, 64], f32, kind="ExternalInput").ap()
    b2 = nc.dram_tensor("b2", [64, 1], f32, kind="ExternalInput").ap()
    w3t = nc.dram_tensor("w3t", [64, H], f32, kind="ExternalInput").ap()
    b3 = nc.dram_tensor("b3", [H, 1], f32, kind="ExternalInput").ap()
    wiha = nc.dram_tensor("wiha", [H + 1, G], f32, kind="ExternalInput").ap()
    whht = nc.dram_tensor("whht", [H, G], f32, kind="ExternalInput").ap()
    wac = nc.dram_tensor("wac", [H, AO], f32, kind="ExternalInput").ap()
    bac = nc.dram_tensor("bac", [AO, 1], f32, kind="ExternalInput").ap()
    ident = nc.dram_tensor("ident", [128, 128], f32, kind="ExternalInput").ap()

    out = nc.dram_tensor("out", [N, AO], f32, kind="ExternalOutput").ap()
    hn = nc.dram_tensor("hn", [BL, H], f32, kind="ExternalOutput").ap()
    cn = nc.dram_tensor("cn", [BL, H], f32, kind="ExternalOutput").ap()

    Sig = mybir.ActivationFunctionType.Sigmoid
    Tanh = mybir.ActivationFunctionType.Tanh
    Ident = mybir.ActivationFunctionType.Identity
    mult = mybir.AluOpType.mult
    add = mybir.AluOpType.add

    XGRP = 8                     # K-tiles per x DMA group
    NGRP = (KT - 1) // XGRP      # 8 full groups; 1 leftover K-tile

    with tile.TileContext(nc) as tc:
        with (
            tc.tile_pool(name="wp", bufs=1) as wp,
            tc.tile_pool(name="xp", bufs=4) as xp,
            tc.tile_pool(name="ap", bufs=2) as ap_,
            tc.tile_pool(name="h3p", bufs=3) as h3p,
            tc.tile_pool(name="sp", bufs=3) as sp,
            tc.tile_pool(name="pl1", bufs=2, space="PSUM") as pl1,
            tc.tile_pool(name="p23", bufs=1, space="PSUM") as p23,
            tc.tile_pool(name="pg", bufs=2, space="PSUM") as pgp,
            tc.tile_pool(name="po", bufs=1, space="PSUM") as pop,
            tc.tile_pool(name="ptr", bufs=1, space="PSUM") as ptrp,
        ):
            # ---- one-time loads ----
            w1sb = wp.tile([128, KT * 64], dtx)
            nc.sync.dma_start(w1sb[:], w1[:])
            w2sb = wp.tile([64, 64], f32)
            nc.sync.dma_start(w2sb[:], w2t[:])
            w3sb = wp.tile([64, H], f32)
            nc.sync.dma_start(w3sb[:], w3t[:])
            wihsb = wp.tile([H + 1, G], f32)
            nc.sync.dma_start(wihsb[:], wiha[:])
            whhsb = wp.tile([H, G], f32)
            nc.sync.dma_start(whhsb[:], whht[:])
            wacsb = wp.tile([H, AO], f32)
            nc.sync.dma_start(wacsb[:], wac[:])
            b1sb = wp.tile([64, 1], f32)
            nc.sync.dma_start(b1sb[:], b1[:])
            b2sb = wp.tile([64, 1], f32)
            nc.sync.dma_start(b2sb[:], b2[:])
            b3sb = wp.tile([H, 1], f32)
            nc.sync.dma_start(b3sb[:], b3[:])
            bacsb = wp.tile([AO, 1], f32)
            nc.sync.dma_start(bacsb[:], bac[:])
            idsb = wp.tile([128, 128], f32)
            nc.sync.dma_start(idsb[:], ident[:])
            masksb = wp.tile([BL, T], f32)
            nc.sync.dma_start(masksb[:], maskc[:])
            h0sb = wp.tile([BL, H], f32)
            nc.sync.dma_start(h0sb[:], h0b[:])
            c0sb = wp.tile([BL, H], f32)
            nc.sync.dma_start(c0sb[:], c0b[:])

            for _rep in range(reps):
                # initial masked state (mask of step 0)
                hm_b = sp.tile([BL, H], f32, tag="hmb")
                nc.vector.tensor_scalar(hm_b[:], h0sb[:], masksb[:, 0:1], None, op0=mult)
                hmT = sp.tile([H, BL], f32, tag="hmT")
                nc.vector.transpose(hmT[:], hm_b[:])
                cm = sp.tile([BL, H], f32, tag="cm")
                nc.vector.tensor_scalar(cm[:], c0sb[:], masksb[:, 0:1], None, op0=mult)

                cst = None
                h_t = None
                for ch in range(NCH):
                    col0 = ch * CW
                    # ---- x chunk load (grouped DMAs) ----
                    xg = []
                    for g in range(NGRP):
                        t_ = xp.tile([128, XGRP * CW], dtx, tag="x8")
                        nc.sync.dma_start(
                            t_[:].rearrange("p (a f) -> p a f", a=XGRP),
                            xt[g * XGRP * 128:(g + 1) * XGRP * 128,
                               col0:col0 + CW].rearrange("(a p) f -> p a f", p=128),
                        )
                        xg.append(t_)
                    xl = xp.tile([128, CW], dtx, tag="x1")
                    nc.sync.dma_start(xl[:], xt[NGRP * XGRP * 128:KT * 128, col0:col0 + CW])

                    # ---- encoder layer 1: h1T = tanh(W1 @ xT / 255 + b1) ----
                    ps1 = pl1.tile([64, CW], f32)
                    for k in range(KT):
                        src = (xg[k // XGRP][:, (k % XGRP) * CW:(k % XGRP + 1) * CW]
                               if k < NGRP * XGRP else xl[:])
                        nc.tensor.matmul(
                            ps1[:], w1sb[:, k * 64:(k + 1) * 64], src,
                            start=(k == 0), stop=(k == KT - 1),
                        )
                    h1 = ap_.tile([64, CW], f32, tag="h1")
                    nc.scalar.activation(h1[:], ps1[:], Tanh, bias=b1sb[:], scale=1.0 / 255.0)

                    # ---- layer 2 ----
                    ps2 = p23.tile([64, CW], f32, tag="ps2")
                    nc.tensor.matmul(ps2[:], w2sb[:], h1[:], start=True, stop=True)
                    h2 = ap_.tile([64, CW], f32, tag="h2")
                    nc.scalar.activation(h2[:], ps2[:], Tanh, bias=b2sb[:])

                    # ---- layer 3 (h3 gets a ones row for the gate bias) ----
                    ps3 = p23.tile([H, CW], f32, tag="ps3")
                    nc.tensor.matmul(ps3[:], w3sb[:], h2[:], start=True, stop=True)
                    h3 = h3p.tile([H + 1, CW], f32, tag="h3")
                    nc.scalar.activation(h3[0:H, :], ps3[:], Tanh, bias=b3sb[:])
                    nc.vector.memset(h3[H:H + 1, :], 1.0)

                    # ---- LSTM scan over this chunk's 16 steps (batch-major) ----
                    hseq = h3p.tile([H, CW], f32, tag="hseq")
                    for tl in range(TPC):
                        t_g = ch * TPC + tl
                        bcol = tl * BL
                        psg = pgp.tile([BL, G], f32)
                        nc.tensor.matmul(psg[:], h3[:, bcol:bcol + BL], wihsb[:],
                                         start=True, stop=False)
                        nc.tensor.matmul(psg[:], hmT[:], whhsb[:],
                                         start=False, stop=True)
                        gact = sp.tile([BL, G], f32, tag="gact")
                        nc.scalar.activation(gact[:, 0:96], psg[:, 0:96], Sig)
                        nc.scalar.activation(gact[:, 96:128], psg[:, 96:128], Tanh)
                        # free-dim gate blocks: i 0:32, f 32:64, o 64:96, g~ 96:128
                        q = sp.tile([BL, H], f32, tag="q")
                        nc.vector.tensor_tensor(q[:], gact[:, 32:64], cm[:], op=mult)
                        p2_ = sp.tile([BL, H], f32, tag="p2")
                        nc.vector.tensor_tensor(p2_[:], gact[:, 0:32], gact[:, 96:128],
                                                op=mult)
                        cst = sp.tile([BL, H], f32, tag="cst")
                        nc.vector.tensor_tensor(cst[:], q[:], p2_[:], op=add)
                        tc_ = sp.tile([BL, H], f32, tag="tc")
                        nc.scalar.activation(tc_[:], cst[:], Tanh)
                        h_t = sp.tile([BL, H], f32, tag="ht")
                        nc.vector.tensor_tensor(h_t[:], gact[:, 64:96], tc_[:], op=mult)
                        nc.vector.transpose(hseq[:, bcol:bcol + BL], h_t[:])
                        if t_g < T - 1:
                            mcol = t_g + 1
                            # hm = (tanh(c) * m) * o  == h_t * m, skipping h_t dep
                            hm_b = sp.tile([BL, H], f32, tag="hmb")
                            nc.vector.scalar_tensor_tensor(
                                hm_b[:], tc_[:], masksb[:, mcol:mcol + 1],
                                gact[:, 64:96], op0=mult, op1=mult)
                            hmT = sp.tile([H, BL], f32, tag="hmT")
                            nc.vector.transpose(hmT[:], hm_b[:])
                            cm = sp.tile([BL, H], f32, tag="cm")
                            nc.vector.tensor_scalar(cm[:], cst[:],
                                                    masksb[:, mcol:mcol + 1], None,
                                                    op0=mult)

                    # ---- head: outT = Wac @ hseq + bac, then transpose out ----
                    pso = pop.tile([AO, CW], f32)
                    nc.tensor.matmul(pso[:], wacsb[:], hseq[:], start=True, stop=True)
                    ot = ap_.tile([AO, CW], f32, tag="ot")
                    nc.scalar.activation(ot[:], pso[:], Ident, bias=bacsb[:])
                    for j in range(CW // 128):
                        ptr_ = ptrp.tile([128, AO], f32, tag="ptr")
                        nc.tensor.transpose(ptr_[:], ot[:, j * 128:(j + 1) * 128],
                                            idsb[0:AO, 0:AO])
                        orow = sp.tile([128, AO], f32, tag="orow")
                        nc.vector.tensor_copy(orow[:], ptr_[:])
                        nc.sync.dma_start(out[col0 + j * 128:col0 + (j + 1) * 128, :],
                                          orow[:])

                # ---- final state hN (last h, unmasked), cN (last c, unmasked) ----
                # batch-major already -- direct DMA out
                nc.sync.dma_start(hn[:], h_t[:])
                nc.sync.dma_start(cn[:], cst[:])

    nc.compile()
    return nc


@functools.lru_cache(maxsize=4)
def _get_program(dt_x: str, reps: int = 1):
    return _build(dt_x, reps)


_GPERM = np.concatenate([np.arange(0, 32), np.arange(32, 64),
                         np.arange(96, 128), np.arange(64, 96)])  # i,f,o,g


def prepare_inputs(x, done, h0, c0, W1, b1, W2, b2, W3, b3,
                   Wih, Whh, bih, bhh, Wa, ba, Wc, bc, dt_x=DT_X):
    """Host-side shard + repack. Returns in_maps for the 8 cores."""
    npdt = _DTX_NP[dt_x]
    x3 = np.asarray(x).reshape(T, B, OBS)
    dn = np.asarray(done).reshape(T, B)

    w1p_flat = np.zeros((OBSP, 64), dtype=npdt)
    w1p_flat[:OBS, :] = np.asarray(W1).T.astype(npdt)
    # pack into the SBUF tile layout [128, KT*64]: w1p[p, 64k+j] = W1T_pad[128k+p, j]
    w1p = np.ascontiguousarray(
        w1p_flat.reshape(KT, 128, 64).transpose(1, 0, 2).reshape(128, KT * 64))
    w2tp = np.ascontiguousarray(np.asarray(W2).T, dtype=np.float32)
    w3tp = np.ascontiguousarray(np.asarray(W3).T, dtype=np.float32)
    wih_p = np.asarray(Wih)[_GPERM]                      # [G, H]
    bg_p = (np.asarray(bih) + np.asarray(bhh))[_GPERM]   # [G]
    wiha = np.concatenate([wih_p.T, bg_p[None, :]], axis=0)  # [H+1, G]
    wiha = np.ascontiguousarray(wiha, dtype=np.float32)
    whh_p = np.ascontiguousarray(np.asarray(Whh)[_GPERM].T, dtype=np.float32)
    wac_p = np.ascontiguousarray(
        np.concatenate([np.asarray(Wa), np.asarray(Wc)], axis=0).T, dtype=np.float32)
    bac_p = np.ascontiguousarray(
        np.concatenate([np.asarray(ba), np.asarray(bc)])[:, None], dtype=np.float32)
    shared = dict(
        w1=w1p, b1=np.asarray(b1, np.float32).reshape(64, 1),
        w2t=w2tp, b2=np.asarray(b2, np.float32).reshape(64, 1),
        w3t=w3tp, b3=np.asarray(b3, np.float32).reshape(H, 1),
        wiha=wiha, whht=whh_p, wac=wac_p, bac=bac_p,
        ident=np.eye(128, dtype=np.float32),
    )

    in_maps = []
    for k in range(NCORES):
        bsl = slice(k * BL, (k + 1) * BL)
        xk = x3[:, bsl, :].reshape(N, OBS)
        xtk = np.zeros((OBSP, N), dtype=npdt)
        xtk[:OBS, :] = xk.T
        maskk = np.ascontiguousarray(1.0 - dn[:, bsl].T, dtype=np.float32)  # [BL, T]
        h0k = np.ascontiguousarray(np.asarray(h0)[0, bsl, :], dtype=np.float32)
        c0k = np.ascontiguousarray(np.asarray(c0)[0, bsl, :], dtype=np.float32)
        in_maps.append(dict(xt=xtk, maskc=maskk, h0b=h0k, c0b=c0k, **shared))
    return in_maps


def gather_outputs(results):
    """Reassemble full outputs from the 8 per-core result dicts."""
    out = np.empty((T, B, AO), dtype=np.float32)
    hN = np.empty((1, B, H), dtype=np.float32)
    cN = np.empty((1, B, H), dtype=np.float32)
    for k in range(NCORES):
        bsl = slice(k * BL, (k + 1) * BL)
        out[:, bsl, :] = results[k]["out"].reshape(T, BL, AO)
        hN[0, bsl, :] = results[k]["hn"]
        cN[0, bsl, :] = results[k]["cn"]
    return out.reshape(T * B, AO), hN, cN


def kernel(**inputs):
    nc = _get_program(DT_X)
    in_maps = prepare_inputs(**inputs, dt_x=DT_X)
    res = run_bass_kernel_spmd(nc, in_maps, list(range(NCORES)))
    return gather_outputs(res.results)
